# revision 31
# baseline (speedup 1.0000x reference)
"""Deformable convolution (B=4, C=256, 64x64, COUT=256, 3x3) on 8 trn2 NeuronCores.

Sharding: data-parallel over (batch, output-row-half): core i handles batch i//2,
output rows [32*(i%2), 32*(i%2)+32). Weight replicated.

Device pipeline per core:
  1. index/fraction math from offsets (DVE, fp32)
  2. one dma_gather per (tap, n-chunk) from a host-staged quad image Q in
     DRAM: row (y, x) = channel-major 2x2 patch [c][TL BL TR BR] (fp16,
     zero-padded borders), 2KB per gathered element
  3. bilinear combine on DVE via scalar_tensor_tensor ops laid out for the
     2x/4x 16-bit fast paths (innermost unit-stride on every operand)
  4. DMA-crossbar transpose (InstDmaTransposeAnt) [n,c] -> [c,n] chunks,
     PE untouched
  5. fp16 GEMM (K=2304) accumulating in PSUM, fp32 output
"""

import os
import sys

for _p in ("/root/.axon_site", "/root/.axon_site/_ro/trn_rl_repo", "/opt/trn_rl_repo"):
    if os.path.isdir(_p) and _p not in sys.path:
        sys.path.append(_p)

import numpy as np

import concourse.bass as bass
import concourse.bacc as bacc
import concourse.mybir as mybir
from concourse.tile import TileContext

# ---------------------------------------------------------------- constants
B, CIN, H, W = 4, 256, 64, 64
COUT, KH, KW = 256, 3, 3
KK = KH * KW                      # 9 taps
HO = WO = 64
HOH = 32                          # output rows per core
N = HOH * WO                      # 2048 positions per core
NJ = 16                           # 128-blocks of N
NCH = 2                           # gather chunks (h)
NJH = NJ // NCH                   # j' blocks per chunk = 8
NIDX = NJH * 128                  # 1024 indices per gather
PAD = 2                           # zero-pad border of the staged image
Hp = Wp = H + 2 * PAD             # 68
NROW = Hp * Wp                    # 4624 quad rows
QE = 4 * CIN                      # 1024 elements per quad row
KB = 2 * KK                       # 18 K-blocks of 128
C288 = KK * 2 * NJ                # 288
C144 = KK * NJ                    # 144
FP16 = mybir.dt.float16
FP32 = mybir.dt.float32
I16 = mybir.dt.int16
I32 = mybir.dt.int32
OP = mybir.AluOpType

_MAX_WAITS = 1


def _split_multiwait_instructions(nc):
    """This walrus build rejects >1 sync wait on one instruction ('Too many
    sync wait commands'); hoist extras onto single-wait EventSemaphore
    instructions inserted just before it."""
    fn = nc.m.functions[0]
    for bb in fn.blocks:
        new_insts = []
        for inst in bb.instructions:
            si = getattr(inst, "sync_info", None)
            if si is not None and si.on_wait and len(si.on_wait) > _MAX_WAITS:
                waits = list(si.on_wait)
                for k, w in enumerate(waits[_MAX_WAITS:]):
                    ev = mybir.InstEventSemaphore(
                        name=f"{inst.name}_wsplit{k}",
                        ins=[],
                        outs=[],
                        sync_info=mybir.SyncInfo(on_wait=[w], on_update=[]),
                    )
                    ev.engine = inst.engine
                    new_insts.append(ev)
                si.on_wait = waits[:_MAX_WAITS]
            new_insts.append(inst)
        bb.instructions[:] = new_insts


# ---------------------------------------------------------------- device kernel
def build_nc(split_waits=True):
    nc = bacc.Bacc()
    img = nc.dram_tensor("img", [NROW, QE], FP16, kind="ExternalInput")
    offg = nc.dram_tensor("offg", [128, C288], FP32, kind="ExternalInput")
    offg16 = nc.dram_tensor("offg16", [16, C288 * 8], FP32, kind="ExternalInput")
    wT = nc.dram_tensor("wT", [KB * 128, COUT], FP16, kind="ExternalInput")
    out = nc.dram_tensor("out", [COUT, N], FP32, kind="ExternalOutput")

    img_src = bass.AP(img[:].tensor, 0, [[QE, NROW], [1, QE]])

    with TileContext(nc) as tc:
        with (
            tc.tile_pool(name="const", bufs=1) as constp,
            tc.tile_pool(name="small", bufs=1) as smallp,
            tc.tile_pool(name="gath", bufs=4) as gathp,
            tc.tile_pool(name="prod", bufs=2) as prodp,
            tc.tile_pool(name="interp", bufs=3) as vp,
            tc.tile_pool(name="cols", bufs=2) as colsp,
            tc.tile_pool(name="osb", bufs=1) as osbp,
            tc.tile_pool(name="pout", bufs=1, space="PSUM") as poutp,
        ):
            # ---- constants (wT host-prearranged to [p, kb, o]; contiguous load)
            w_sb = constp.tile([128, KB, COUT], FP16)
            nc.gpsimd.dma_start(w_sb[:], wT[:].rearrange("(p kb) o -> p kb o",
                                                         p=128))

            # ---- stage A: sampling positions (host pre-adds grid to the
            # offsets: pp = offset + base grid + 16 bias), fractions, weights
            pp = smallp.tile([128, C288], FP32, tag="pp")
            nc.gpsimd.dma_start(pp[:], offg[:])
            # floor(pp): int-cast rounds-to-nearest on HW; correct to floor
            # via (cast > pp) ? cast-1 : cast.
            p_i = smallp.tile([128, C288], I32, tag="pi")
            nc.vector.tensor_copy(out=p_i[:], in_=pp[:])
            p_f = smallp.tile([128, C288], FP32, tag="pf")
            nc.vector.tensor_copy(out=p_f[:], in_=p_i[:])
            gt_t = smallp.tile([128, C288], FP32, tag="gtt")
            nc.vector.tensor_tensor(out=gt_t[:], in0=p_f[:], in1=pp[:],
                                    op=OP.is_gt)
            nc.vector.tensor_tensor(out=p_f[:], in0=p_f[:], in1=gt_t[:],
                                    op=OP.subtract)
            fr = smallp.tile([128, C288], FP32, tag="fr")
            nc.vector.tensor_tensor(out=fr[:], in0=pp[:], in1=p_f[:],
                                    op=OP.subtract)
            omfr = smallp.tile([128, C288], FP32, tag="omfr")
            nc.vector.tensor_scalar(out=omfr[:], in0=fr[:], scalar1=-1.0,
                                    scalar2=1.0, op0=OP.mult, op1=OP.add)

            # per-tap (k, d, j) views: y = d0, x = d1 -> [128, 9, 16]
            def yx(t):
                v4 = t[:].rearrange("p (k d j) -> p k d j", d=2, j=NJ)
                return v4[:, :, 0, :], v4[:, :, 1, :]

            fr_y, fr_x = yx(fr)
            om_y, om_x = yx(omfr)

            # bilinear weights -> w4 [128, (k j q)] fp16, q order (TL,BL,TR,BR)
            w4 = smallp.tile([128, C144 * 4], FP16, tag="w4")
            w4v = w4[:].rearrange("p (k j q) -> p k j q", k=KK, j=NJ)
            nc.vector.tensor_tensor(out=w4v[:, :, :, 0], in0=om_y, in1=om_x,
                                    op=OP.mult)  # TL: (1-ly)(1-lx)
            nc.vector.tensor_tensor(out=w4v[:, :, :, 1], in0=fr_y, in1=om_x,
                                    op=OP.mult)  # BL: ly(1-lx)
            nc.vector.tensor_tensor(out=w4v[:, :, :, 2], in0=om_y, in1=fr_x,
                                    op=OP.mult)  # TR: (1-ly)lx
            nc.vector.tensor_tensor(out=w4v[:, :, :, 3], in0=fr_y, in1=fr_x,
                                    op=OP.mult)  # BR: ly lx

            # indices (y0,x0 carry a +16 bias from the host grid):
            # idx = clamp(y0-14,0,67)*68 + clamp(x0-14,0,67), computed
            # directly in the gather ucode's folded [16-partition, (k j r)]
            # layout (idx i of a gather lives at [i%16, i//16]) from the
            # host-prepped offg16 (= offsets + grid + bias) — avoids
            # 2-byte-element fold DMAs.
            pp6 = smallp.tile([16, C288 * 8], FP32, tag="pp6")
            nc.gpsimd.dma_start(pp6[:], offg16[:])
            pi6 = smallp.tile([16, C288 * 8], I32, tag="scr6")
            nc.vector.tensor_copy(out=pi6[:], in_=pp6[:])
            pf6 = smallp.tile([16, C288 * 8], FP32, tag="pf6")
            nc.vector.tensor_copy(out=pf6[:], in_=pi6[:])
            gt6 = smallp.tile([16, C288 * 8], FP32, tag="scr6",
                              name="gt6")
            nc.vector.tensor_tensor(out=gt6[:], in0=pf6[:], in1=pp6[:],
                                    op=OP.is_gt)
            nc.vector.tensor_tensor(out=pf6[:], in0=pf6[:], in1=gt6[:],
                                    op=OP.subtract)
            pf6v = pf6[:].rearrange("p (k d a) -> p k d a", d=2, a=NJ * 8)
            # tt/ss packed in one scratch tile (reuses the pi6/gt6 ring slot)
            ts6 = smallp.tile([16, C288 * 8], FP32, tag="scr6", name="ts6")
            tt6 = ts6[:, 0:C144 * 8]
            ss6 = ts6[:, C144 * 8:]
            t6 = tt6.rearrange("p (k a) -> p k a", a=NJ * 8)
            s6 = ss6.rearrange("p (k a) -> p k a", a=NJ * 8)
            nc.vector.tensor_scalar(out=t6, in0=pf6v[:, :, 0, :],
                                    scalar1=-14.0, scalar2=0.0,
                                    op0=OP.add, op1=OP.max)
            nc.vector.tensor_scalar(out=tt6, in0=tt6, scalar1=67.0,
                                    scalar2=float(Wp), op0=OP.min,
                                    op1=OP.mult)
            nc.vector.tensor_scalar(out=s6, in0=pf6v[:, :, 1, :],
                                    scalar1=-14.0, scalar2=0.0,
                                    op0=OP.add, op1=OP.max)
            nc.vector.tensor_scalar(out=ss6, in0=ss6, scalar1=67.0,
                                    scalar2=None, op0=OP.min)
            idxf6c = smallp.tile([16, C288 * 8], FP32, tag="pf6",
                                 name="idxf6c")
            idxf6 = idxf6c[:, 0:C144 * 8]
            nc.vector.tensor_tensor(out=idxf6, in0=tt6, in1=ss6,
                                    op=OP.add)
            idx16 = constp.tile([128, 8 * C144], I16)
            nc.vector.tensor_copy(out=idx16[0:16, :], in_=idxf6)
            for g in range(1, 8):
                nc.gpsimd.dma_start(out=idx16[g * 16:(g + 1) * 16, :],
                                    in_=idx16[0:16, :])

            # ---- stages B-E, software-pipelined: each chunk's gather+mult
            # issue first (the mult frees the g ring slot, gating the gather
            # 4 ahead); the previous chunk's s/v trail; transposes+matmuls
            # run once per KBATCH chunks (few HWDGE DMAs -> no semaphore-lane
            # recycling stalls on the gathers).
            KBATCH = 3
            pout_by_h = {}
            state = {"v": None, "n": 0}
            pending = None

            def flush_batch(h, klast):
                vbig, n = state["v"], state["n"]
                state["v"], state["n"] = None, 0
                pout = pout_by_h[h]
                vt = colsp.tile([128, KBATCH * 16, 128], FP16, tag="vt")
                nc.sync.dma_start_transpose(
                    vt[:], vbig[:].rearrange("p a b c d -> p (a b c) d"))
                for i in range(n):
                    k = klast - n + 1 + i
                    for cb in range(2):
                        kb = k * 2 + cb
                        for ob in range(2):
                            for ns in range(2):
                                nc.tensor.matmul(
                                    pout[ob][:, ns * 512:(ns + 1) * 512],
                                    lhsT=w_sb[:, kb, ob * 128:(ob + 1) * 128],
                                    rhs=vt[:, i * 16 + cb * 8 + ns * 4:
                                           i * 16 + cb * 8 + (ns + 1) * 4, :],
                                    start=(kb == 0), stop=(kb == KB - 1))

            def finish(ch):
                h, k, prods = ch
                if h not in pout_by_h:
                    pout_by_h[h] = [
                        poutp.tile([128, NIDX], FP32, tag=f"pout{ob}",
                                   name=f"pout{ob}_{h}")
                        for ob in range(2)]
                pv = prods[:].rearrange("p a c (y x) -> p a c y x", y=2)
                # pair-add in place over the first half of prods (safe:
                # each written element is read before any later one needs it)
                s = pv[:, :, :, 0, :]
                nc.vector.tensor_tensor(
                    out=s, in0=pv[:, :, :, 0, :],
                    in1=pv[:, :, :, 1, :], op=OP.add)
                # final add written (cb, j, c')-major into the batch buffer
                # so GEMM rhs slices are contiguous after the transpose
                if state["v"] is None:
                    state["v"] = vp.tile([128, KBATCH, 2, NJH, 128], FP16,
                                         tag="v", name=f"vbig_{h}_{k}")
                vbig = state["v"]
                i = state["n"]
                vs = vbig[:, i]
                vw = bass.AP(vs.tensor, vs.offset,
                             [vs.ap[0], [128, NJH], [1024, 2], [1, 128]])
                # s lives in prods[..., 0:2] (strides: a 1024, cb 512, c' 4)
                pt = prods[:]
                sv0 = bass.AP(pt.tensor, pt.offset,
                              [pt.ap[0], [1024, NJH], [512, 2], [4, 128]])
                sv1 = bass.AP(pt.tensor, pt.offset + 1,
                              [pt.ap[0], [1024, NJH], [512, 2], [4, 128]])
                nc.vector.tensor_tensor(out=vw, in0=sv0, in1=sv1, op=OP.add)
                state["n"] = i + 1
                if state["n"] == KBATCH or k == KK - 1:
                    flush_batch(h, k)
                if k == KK - 1:
                    pout = pout_by_h[h]
                    for ob in range(2):
                        osb = osbp.tile([128, NIDX], FP32, tag="osb")
                        nc.scalar.copy(out=osb[:], in_=pout[ob][:])
                        nc.scalar.dma_start(
                            out=out[ob * 128:(ob + 1) * 128,
                                    h * NIDX:(h + 1) * NIDX],
                            in_=osb[:])

            for h in range(NCH):
                for k in range(KK):
                    g = gathp.tile([128, NJH, QE], FP16, tag="g")
                    base = (k * NJ + h * NJH) * 8
                    nc.gpsimd.dma_gather(
                        g[:], img_src, idx16[:, base:base + NIDX // 16],
                        NIDX, NIDX, QE)

                    # bilinear products, all operands innermost unit-stride
                    g4 = g[:].rearrange("p a (c q) -> p a c q", q=4)
                    wv = w4[:, (k * NJ + h * NJH) * 4:]
                    wb = bass.AP(wv.tensor, wv.offset,
                                 [wv.ap[0], [4, NJH], [0, CIN], [1, 4]])
                    prods = prodp.tile([128, NJH, CIN, 4], FP16, tag="prods")
                    nc.vector.tensor_tensor(out=prods[:], in0=g4, in1=wb,
                                            op=OP.mult)
                    if pending is not None:
                        finish(pending)
                    pending = (h, k, prods)
            finish(pending)

    nc.compile()
    if split_waits:
        _split_multiwait_instructions(nc)
    return nc


_NC_CACHE = None


def _get_nc():
    global _NC_CACHE
    if _NC_CACHE is None:
        _NC_CACHE = build_nc()
    return _NC_CACHE


# ---------------------------------------------------------------- host prep
def _prep_core_inputs(x, offset, weight):
    """Build the 8 per-core input maps (pure layout/pad/cast transforms)."""
    x = np.asarray(x, np.float32)
    offset = np.asarray(offset, np.float32)
    weight = np.asarray(weight, np.float32)

    imgs = []
    for b in range(B):
        pimg = np.zeros((Hp + 1, Wp + 1, CIN), np.float16)
        pimg[PAD:PAD + H, PAD:PAD + W, :] = x[b].transpose(1, 2, 0)
        # quad rows: row (y, x) = [c][TL BL TR BR] channel-major
        quad = np.stack([pimg[:Hp, :Wp], pimg[1:Hp + 1, :Wp],
                         pimg[:Hp, 1:Wp + 1], pimg[1:Hp + 1, 1:Wp + 1]],
                        axis=-1)  # [Hp, Wp, CIN, 4]
        imgs.append(np.ascontiguousarray(quad.reshape(NROW, QE)))

    # [p, kb, o] layout so the device load is one contiguous DMA
    wT = (weight.transpose(2, 3, 1, 0).reshape(KB, 128, COUT)
          .transpose(1, 0, 2).reshape(KB * 128, COUT))
    wT = np.ascontiguousarray(wT.astype(np.float16))

    # base grid (+16 bias for floor correction): cols (k, d, j), n = j*128+p
    p = np.arange(128)
    j = np.arange(NJ)
    n = j[None, :] * 128 + p[:, None]          # [128, 16]
    grids = []
    for half in range(2):
        ho0 = half * HOH
        g = np.empty((128, KK, 2, NJ), np.float32)
        for kh in range(KH):
            for kw in range(KW):
                k = kh * KW + kw
                g[:, k, 0, :] = kh + (ho0 + n // WO) - 1 + 16
                g[:, k, 1, :] = kw + (n % WO) - 1 + 16
        grids.append(np.ascontiguousarray(g.reshape(128, C288)))

    in_maps = []
    for core in range(8):
        b, half = core // 2, core % 2
        ho0 = half * HOH
        offc = offset[b].reshape(KK, 2, HO, WO)[:, :, ho0:ho0 + HOH, :]
        offc = offc.reshape(KK, 2, NJ, 128)          # [k, d, j, p]
        offg_np = (offc.transpose(3, 0, 1, 2).reshape(128, C288)
                   + grids[half])
        offg_np = np.ascontiguousarray(offg_np)
        # folded [16, (k d j r)] layout: value for position n = j*128+r*16+q
        # at [q, (k, d, j, r)]
        a = offg_np.reshape(8, 16, KK, 2, NJ)        # [r, q, k, d, j]
        offg16_np = np.ascontiguousarray(
            a.transpose(1, 2, 3, 4, 0).reshape(16, C288 * 8))
        in_maps.append({
            "img": imgs[b],
            "offg": offg_np,
            "offg16": offg16_np,
            "wT": wT,
        })
    return in_maps


def _assemble(results):
    out = np.empty((B, COUT, HO, WO), np.float32)
    for core, r in enumerate(results):
        b, half = core // 2, core % 2
        out[b, :, half * HOH:(half + 1) * HOH, :] = (
            r["out"].reshape(COUT, HOH, WO))
    return out


def kernel(x, offset, weight):
    from concourse.bass_utils import run_bass_kernel_spmd

    nc = _get_nc()
    in_maps = _prep_core_inputs(x, offset, weight)
    res = run_bass_kernel_spmd(nc, in_maps, core_ids=list(range(8)))
    return _assemble(res.results)


# revision 32
# speedup vs baseline: 1.2048x; 1.2048x over previous
"""Deformable convolution (B=4, C=256, 64x64, COUT=256, 3x3) on 8 trn2 NeuronCores.

Sharding: data-parallel over (batch, output-row-half): core i handles batch i//2,
output rows [32*(i%2), 32*(i%2)+32). Weight replicated.

Device pipeline per core:
  1. index/fraction math from offsets (DVE, fp32)
  2. one dma_gather per (tap, n-chunk) from a host-staged quad image Q in
     DRAM: row (y, x) = channel-major 2x2 patch [c][TL BL TR BR] (fp16,
     zero-padded borders), 2KB per gathered element
  3. bilinear combine on DVE via scalar_tensor_tensor ops laid out for the
     2x/4x 16-bit fast paths (innermost unit-stride on every operand)
  4. DMA-crossbar transpose (InstDmaTransposeAnt) [n,c] -> [c,n] chunks,
     PE untouched
  5. fp16 GEMM (K=2304) accumulating in PSUM, fp32 output
"""

import os
import sys

for _p in ("/root/.axon_site", "/root/.axon_site/_ro/trn_rl_repo", "/opt/trn_rl_repo"):
    if os.path.isdir(_p) and _p not in sys.path:
        sys.path.append(_p)

import numpy as np

import concourse.bass as bass
import concourse.bacc as bacc
import concourse.mybir as mybir
from concourse.tile import TileContext

# ---------------------------------------------------------------- constants
B, CIN, H, W = 4, 256, 64, 64
COUT, KH, KW = 256, 3, 3
KK = KH * KW                      # 9 taps
HO = WO = 64
HOH = 32                          # output rows per core
N = HOH * WO                      # 2048 positions per core
NJ = 16                           # 128-blocks of N
NCH = 2                           # gather chunks (h)
NJH = NJ // NCH                   # j' blocks per chunk = 8
NIDX = NJH * 128                  # 1024 indices per gather
PAD = 2                           # zero-pad border of the staged image
Hp = Wp = H + 2 * PAD             # 68
NROW = Hp * Wp                    # 4624 quad rows
QE = 4 * CIN                      # 1024 elements per quad row
KB = 2 * KK                       # 18 K-blocks of 128
C288 = KK * 2 * NJ                # 288
C144 = KK * NJ                    # 144
FP16 = mybir.dt.float16
FP32 = mybir.dt.float32
I16 = mybir.dt.int16
I32 = mybir.dt.int32
OP = mybir.AluOpType

_MAX_WAITS = 1


def _strip_pool_hwdge_guards(nc):
    """Remove DMAHW-lane waits from Pool-engine EventSemaphore guards.

    The tile scheduler hoists conservative engine-clock catch-up waits onto
    the gpsimd stream before each gather; their DMAHW components reference
    recent crossbar transposes the gathers have no true dependency on (the
    real vbig WAR waits are carried by the DVE ops themselves), and they
    serialize the gather pipeline."""
    fn = nc.m.functions[0]
    for bb in fn.blocks:
        for inst in bb.instructions:
            if not isinstance(inst, mybir.InstEventSemaphore):
                continue
            if inst.engine != mybir.EngineType.Pool:
                continue
            si = getattr(inst, "sync_info", None)
            if si is None or not si.on_wait:
                continue
            si.on_wait = [w for w in si.on_wait
                          if not (w.ant_name or "").startswith("DMAHW")]


def _split_multiwait_instructions(nc):
    """This walrus build rejects >1 sync wait on one instruction ('Too many
    sync wait commands'); hoist extras onto single-wait EventSemaphore
    instructions inserted just before it."""
    fn = nc.m.functions[0]
    for bb in fn.blocks:
        new_insts = []
        for inst in bb.instructions:
            si = getattr(inst, "sync_info", None)
            if si is not None and si.on_wait and len(si.on_wait) > _MAX_WAITS:
                waits = list(si.on_wait)
                for k, w in enumerate(waits[_MAX_WAITS:]):
                    ev = mybir.InstEventSemaphore(
                        name=f"{inst.name}_wsplit{k}",
                        ins=[],
                        outs=[],
                        sync_info=mybir.SyncInfo(on_wait=[w], on_update=[]),
                    )
                    ev.engine = inst.engine
                    new_insts.append(ev)
                si.on_wait = waits[:_MAX_WAITS]
            new_insts.append(inst)
        bb.instructions[:] = new_insts


# ---------------------------------------------------------------- device kernel
def build_nc(split_waits=True):
    nc = bacc.Bacc()
    img = nc.dram_tensor("img", [NROW, QE], FP16, kind="ExternalInput")
    offg = nc.dram_tensor("offg", [128, C288], FP32, kind="ExternalInput")
    offg16 = nc.dram_tensor("offg16", [16, C288 * 8], FP32, kind="ExternalInput")
    wT = nc.dram_tensor("wT", [KB * 128, COUT], FP16, kind="ExternalInput")
    out = nc.dram_tensor("out", [COUT, N], FP32, kind="ExternalOutput")

    img_src = bass.AP(img[:].tensor, 0, [[QE, NROW], [1, QE]])

    with TileContext(nc) as tc:
        with (
            tc.tile_pool(name="const", bufs=1) as constp,
            tc.tile_pool(name="small", bufs=1) as smallp,
            tc.tile_pool(name="gath", bufs=4) as gathp,
            tc.tile_pool(name="prod", bufs=2) as prodp,
            tc.tile_pool(name="interp", bufs=2) as vp,
            tc.tile_pool(name="cols", bufs=2) as colsp,
            tc.tile_pool(name="osb", bufs=1) as osbp,
            tc.tile_pool(name="pout", bufs=1, space="PSUM") as poutp,
        ):
            # ---- constants (wT host-prearranged to [p, kb, o]; contiguous load)
            w_sb = constp.tile([128, KB, COUT], FP16)
            nc.gpsimd.dma_start(w_sb[:], wT[:].rearrange("(p kb) o -> p kb o",
                                                         p=128))

            # ---- stage A: sampling positions (host pre-adds grid to the
            # offsets: pp = offset + base grid + 16 bias), fractions, weights
            pp = smallp.tile([128, C288], FP32, tag="pp")
            nc.gpsimd.dma_start(pp[:], offg[:])
            # floor(pp): int-cast rounds-to-nearest on HW; correct to floor
            # via (cast > pp) ? cast-1 : cast.
            p_i = smallp.tile([128, C288], I32, tag="pi")
            nc.vector.tensor_copy(out=p_i[:], in_=pp[:])
            p_f = smallp.tile([128, C288], FP32, tag="pf")
            nc.vector.tensor_copy(out=p_f[:], in_=p_i[:])
            gt_t = smallp.tile([128, C288], FP32, tag="gtt")
            nc.vector.tensor_tensor(out=gt_t[:], in0=p_f[:], in1=pp[:],
                                    op=OP.is_gt)
            nc.vector.tensor_tensor(out=p_f[:], in0=p_f[:], in1=gt_t[:],
                                    op=OP.subtract)
            fr = smallp.tile([128, C288], FP32, tag="fr")
            nc.vector.tensor_tensor(out=fr[:], in0=pp[:], in1=p_f[:],
                                    op=OP.subtract)
            omfr = smallp.tile([128, C288], FP32, tag="omfr")
            nc.vector.tensor_scalar(out=omfr[:], in0=fr[:], scalar1=-1.0,
                                    scalar2=1.0, op0=OP.mult, op1=OP.add)

            # per-tap (k, d, j) views: y = d0, x = d1 -> [128, 9, 16]
            def yx(t):
                v4 = t[:].rearrange("p (k d j) -> p k d j", d=2, j=NJ)
                return v4[:, :, 0, :], v4[:, :, 1, :]

            fr_y, fr_x = yx(fr)
            om_y, om_x = yx(omfr)

            # bilinear weights -> w4 [128, (k j q)] fp16, q order (TL,BL,TR,BR)
            w4 = smallp.tile([128, C144 * 4], FP16, tag="w4")
            w4v = w4[:].rearrange("p (k j q) -> p k j q", k=KK, j=NJ)
            nc.vector.tensor_tensor(out=w4v[:, :, :, 0], in0=om_y, in1=om_x,
                                    op=OP.mult)  # TL: (1-ly)(1-lx)
            nc.vector.tensor_tensor(out=w4v[:, :, :, 1], in0=fr_y, in1=om_x,
                                    op=OP.mult)  # BL: ly(1-lx)
            nc.vector.tensor_tensor(out=w4v[:, :, :, 2], in0=om_y, in1=fr_x,
                                    op=OP.mult)  # TR: (1-ly)lx
            nc.vector.tensor_tensor(out=w4v[:, :, :, 3], in0=fr_y, in1=fr_x,
                                    op=OP.mult)  # BR: ly lx

            # indices (y0,x0 carry a +16 bias from the host grid):
            # idx = clamp(y0-14,0,67)*68 + clamp(x0-14,0,67), computed
            # directly in the gather ucode's folded [16-partition, (k j r)]
            # layout (idx i of a gather lives at [i%16, i//16]) from the
            # host-prepped offg16 (= offsets + grid + bias) — avoids
            # 2-byte-element fold DMAs.
            pp6 = smallp.tile([16, C288 * 8], FP32, tag="pp6")
            nc.gpsimd.dma_start(pp6[:], offg16[:])
            pi6 = smallp.tile([16, C288 * 8], I32, tag="scr6")
            nc.vector.tensor_copy(out=pi6[:], in_=pp6[:])
            pf6 = smallp.tile([16, C288 * 8], FP32, tag="pf6")
            nc.vector.tensor_copy(out=pf6[:], in_=pi6[:])
            gt6 = smallp.tile([16, C288 * 8], FP32, tag="scr6",
                              name="gt6")
            nc.vector.tensor_tensor(out=gt6[:], in0=pf6[:], in1=pp6[:],
                                    op=OP.is_gt)
            nc.vector.tensor_tensor(out=pf6[:], in0=pf6[:], in1=gt6[:],
                                    op=OP.subtract)
            pf6v = pf6[:].rearrange("p (k d a) -> p k d a", d=2, a=NJ * 8)
            # tt/ss packed in one scratch tile (reuses the pi6/gt6 ring slot)
            ts6 = smallp.tile([16, C288 * 8], FP32, tag="scr6", name="ts6")
            tt6 = ts6[:, 0:C144 * 8]
            ss6 = ts6[:, C144 * 8:]
            t6 = tt6.rearrange("p (k a) -> p k a", a=NJ * 8)
            s6 = ss6.rearrange("p (k a) -> p k a", a=NJ * 8)
            nc.vector.tensor_scalar(out=t6, in0=pf6v[:, :, 0, :],
                                    scalar1=-14.0, scalar2=0.0,
                                    op0=OP.add, op1=OP.max)
            nc.vector.tensor_scalar(out=tt6, in0=tt6, scalar1=67.0,
                                    scalar2=float(Wp), op0=OP.min,
                                    op1=OP.mult)
            nc.vector.tensor_scalar(out=s6, in0=pf6v[:, :, 1, :],
                                    scalar1=-14.0, scalar2=0.0,
                                    op0=OP.add, op1=OP.max)
            nc.vector.tensor_scalar(out=ss6, in0=ss6, scalar1=67.0,
                                    scalar2=None, op0=OP.min)
            idxf6c = smallp.tile([16, C288 * 8], FP32, tag="pf6",
                                 name="idxf6c")
            idxf6 = idxf6c[:, 0:C144 * 8]
            nc.vector.tensor_tensor(out=idxf6, in0=tt6, in1=ss6,
                                    op=OP.add)
            idx16 = constp.tile([128, 8 * C144], I16)
            nc.vector.tensor_copy(out=idx16[0:16, :], in_=idxf6)
            for g in range(1, 8):
                nc.gpsimd.dma_start(out=idx16[g * 16:(g + 1) * 16, :],
                                    in_=idx16[0:16, :])

            # ---- stages B-E, software-pipelined: each chunk's gather+mult
            # issue first (the mult frees the g ring slot, gating the gather
            # 4 ahead); the previous chunk's s/v trail; transposes+matmuls
            # run once per KBATCH chunks (few HWDGE DMAs -> no semaphore-lane
            # recycling stalls on the gathers).
            KBATCH = 3
            pout_by_h = {}
            state = {"v": None, "n": 0}
            pending = None

            def flush_batch(h, klast):
                vbig, n = state["v"], state["n"]
                state["v"], state["n"] = None, 0
                pout = pout_by_h[h]
                vt = colsp.tile([128, KBATCH * 16, 128], FP16, tag="vt")
                nc.sync.dma_start_transpose(
                    vt[:], vbig[:].rearrange("p a b c d -> p (a b c) d"))
                for i in range(n):
                    k = klast - n + 1 + i
                    for cb in range(2):
                        kb = k * 2 + cb
                        for ob in range(2):
                            for ns in range(2):
                                nc.tensor.matmul(
                                    pout[ob][:, ns * 512:(ns + 1) * 512],
                                    lhsT=w_sb[:, kb, ob * 128:(ob + 1) * 128],
                                    rhs=vt[:, i * 16 + cb * 8 + ns * 4:
                                           i * 16 + cb * 8 + (ns + 1) * 4, :],
                                    start=(kb == 0), stop=(kb == KB - 1))

            def finish(ch):
                h, k, prods = ch
                if h not in pout_by_h:
                    pout_by_h[h] = [
                        poutp.tile([128, NIDX], FP32, tag=f"pout{ob}",
                                   name=f"pout{ob}_{h}")
                        for ob in range(2)]
                pv = prods[:].rearrange("p a c (y x) -> p a c y x", y=2)
                # pair-add in place over the first half of prods (safe:
                # each written element is read before any later one needs it)
                s = pv[:, :, :, 0, :]
                nc.vector.tensor_tensor(
                    out=s, in0=pv[:, :, :, 0, :],
                    in1=pv[:, :, :, 1, :], op=OP.add)
                # final add written (cb, j, c')-major into the batch buffer
                # so GEMM rhs slices are contiguous after the transpose
                if state["v"] is None:
                    state["v"] = vp.tile([128, KBATCH, 2, NJH, 128], FP16,
                                         tag="v", name=f"vbig_{h}_{k}")
                vbig = state["v"]
                i = state["n"]
                vs = vbig[:, i]
                vw = bass.AP(vs.tensor, vs.offset,
                             [vs.ap[0], [128, NJH], [1024, 2], [1, 128]])
                # s lives in prods[..., 0:2] (strides: a 1024, cb 512, c' 4)
                pt = prods[:]
                sv0 = bass.AP(pt.tensor, pt.offset,
                              [pt.ap[0], [1024, NJH], [512, 2], [4, 128]])
                sv1 = bass.AP(pt.tensor, pt.offset + 1,
                              [pt.ap[0], [1024, NJH], [512, 2], [4, 128]])
                nc.vector.tensor_tensor(out=vw, in0=sv0, in1=sv1, op=OP.add)
                state["n"] = i + 1
                if state["n"] == KBATCH or k == KK - 1:
                    flush_batch(h, k)
                if k == KK - 1:
                    pout = pout_by_h[h]
                    for ob in range(2):
                        osb = osbp.tile([128, NIDX], FP32, tag="osb")
                        nc.scalar.copy(out=osb[:], in_=pout[ob][:])
                        nc.scalar.dma_start(
                            out=out[ob * 128:(ob + 1) * 128,
                                    h * NIDX:(h + 1) * NIDX],
                            in_=osb[:])

            for h in range(NCH):
                for k in range(KK):
                    g = gathp.tile([128, NJH, QE], FP16, tag="g")
                    base = (k * NJ + h * NJH) * 8
                    nc.gpsimd.dma_gather(
                        g[:], img_src, idx16[:, base:base + NIDX // 16],
                        NIDX, NIDX, QE)

                    # bilinear products, all operands innermost unit-stride
                    g4 = g[:].rearrange("p a (c q) -> p a c q", q=4)
                    wv = w4[:, (k * NJ + h * NJH) * 4:]
                    wb = bass.AP(wv.tensor, wv.offset,
                                 [wv.ap[0], [4, NJH], [0, CIN], [1, 4]])
                    prods = prodp.tile([128, NJH, CIN, 4], FP16, tag="prods")
                    nc.vector.tensor_tensor(out=prods[:], in0=g4, in1=wb,
                                            op=OP.mult)
                    if pending is not None:
                        finish(pending)
                    pending = (h, k, prods)
            finish(pending)

    nc.compile()
    _strip_pool_hwdge_guards(nc)
    if split_waits:
        _split_multiwait_instructions(nc)
    return nc


_NC_CACHE = None


def _get_nc():
    global _NC_CACHE
    if _NC_CACHE is None:
        _NC_CACHE = build_nc()
    return _NC_CACHE


# ---------------------------------------------------------------- host prep
def _prep_core_inputs(x, offset, weight):
    """Build the 8 per-core input maps (pure layout/pad/cast transforms)."""
    x = np.asarray(x, np.float32)
    offset = np.asarray(offset, np.float32)
    weight = np.asarray(weight, np.float32)

    imgs = []
    for b in range(B):
        pimg = np.zeros((Hp + 1, Wp + 1, CIN), np.float16)
        pimg[PAD:PAD + H, PAD:PAD + W, :] = x[b].transpose(1, 2, 0)
        # quad rows: row (y, x) = [c][TL BL TR BR] channel-major
        quad = np.stack([pimg[:Hp, :Wp], pimg[1:Hp + 1, :Wp],
                         pimg[:Hp, 1:Wp + 1], pimg[1:Hp + 1, 1:Wp + 1]],
                        axis=-1)  # [Hp, Wp, CIN, 4]
        imgs.append(np.ascontiguousarray(quad.reshape(NROW, QE)))

    # [p, kb, o] layout so the device load is one contiguous DMA
    wT = (weight.transpose(2, 3, 1, 0).reshape(KB, 128, COUT)
          .transpose(1, 0, 2).reshape(KB * 128, COUT))
    wT = np.ascontiguousarray(wT.astype(np.float16))

    # base grid (+16 bias for floor correction): cols (k, d, j), n = j*128+p
    p = np.arange(128)
    j = np.arange(NJ)
    n = j[None, :] * 128 + p[:, None]          # [128, 16]
    grids = []
    for half in range(2):
        ho0 = half * HOH
        g = np.empty((128, KK, 2, NJ), np.float32)
        for kh in range(KH):
            for kw in range(KW):
                k = kh * KW + kw
                g[:, k, 0, :] = kh + (ho0 + n // WO) - 1 + 16
                g[:, k, 1, :] = kw + (n % WO) - 1 + 16
        grids.append(np.ascontiguousarray(g.reshape(128, C288)))

    in_maps = []
    for core in range(8):
        b, half = core // 2, core % 2
        ho0 = half * HOH
        offc = offset[b].reshape(KK, 2, HO, WO)[:, :, ho0:ho0 + HOH, :]
        offc = offc.reshape(KK, 2, NJ, 128)          # [k, d, j, p]
        offg_np = (offc.transpose(3, 0, 1, 2).reshape(128, C288)
                   + grids[half])
        offg_np = np.ascontiguousarray(offg_np)
        # folded [16, (k d j r)] layout: value for position n = j*128+r*16+q
        # at [q, (k, d, j, r)]
        a = offg_np.reshape(8, 16, KK, 2, NJ)        # [r, q, k, d, j]
        offg16_np = np.ascontiguousarray(
            a.transpose(1, 2, 3, 4, 0).reshape(16, C288 * 8))
        in_maps.append({
            "img": imgs[b],
            "offg": offg_np,
            "offg16": offg16_np,
            "wT": wT,
        })
    return in_maps


def _assemble(results):
    out = np.empty((B, COUT, HO, WO), np.float32)
    for core, r in enumerate(results):
        b, half = core // 2, core % 2
        out[b, :, half * HOH:(half + 1) * HOH, :] = (
            r["out"].reshape(COUT, HOH, WO))
    return out


def kernel(x, offset, weight):
    from concourse.bass_utils import run_bass_kernel_spmd

    nc = _get_nc()
    in_maps = _prep_core_inputs(x, offset, weight)
    res = run_bass_kernel_spmd(nc, in_maps, core_ids=list(range(8)))
    return _assemble(res.results)
